# revision 7
# baseline (speedup 1.0000x reference)
"""Trainium2 Bass kernel for nn_Conv_ImgEncoder (dense_cnn).

Sharding: data-parallel over batch (8 samples -> 8 cores). Each core runs the
full per-sample pipeline: trans_img (mask crop+nearest resize via exact fp32
one-hot matmul gathers) followed by the grouped-conv stack (float32r matmuls,
fp32 PSUM accumulation) with fused InstanceNorm+LeakyReLU epilogues.
"""
import os
import sys
import numpy as np

for _p in ('/opt/trn_rl_repo',):
    if _p not in sys.path:
        sys.path.insert(0, _p)

import concourse.bass as bass
import concourse.tile as tile
from concourse import bacc, mybir
from concourse.bass_utils import run_bass_kernel_spmd

f32 = mybir.dt.float32
F32R = mybir.dt.float32r
CDT = F32R if os.environ.get("BASS_CONV_DTYPE", "f32r") == "f32r" else f32
AF = mybir.ActivationFunctionType
OP = mybir.AluOpType
AX = mybir.AxisListType

B, S, H, W = 8, 8, 256, 256
BIG = float(2 ** 30)
DYDX = [(dy, dx) for dy in range(3) for dx in range(3)]

LAST_EXEC_NS = None
LAST_RESULTS = None
_CACHE = {}


# ---------------------------------------------------------------- host prep
def _sn_np(w, seed):
    o = w.shape[0]
    m = w.reshape(o, -1).astype(np.float32)
    u = np.random.RandomState(seed).randn(o).astype(np.float32)
    u = u / (np.linalg.norm(u) + 1e-12)
    v = m.T @ u
    v = v / (np.linalg.norm(v) + 1e-12)
    u2 = m @ v
    u2 = u2 / (np.linalg.norm(u2) + 1e-12)
    sigma = u2 @ (m @ v)
    return (w / sigma).astype(np.float32)


def _pack_weights(ws):
    w1, w2, w3, w4, w5, w6 = [_sn_np(w, i + 1) for i, w in enumerate(ws)]
    d1 = np.zeros((24, 9, 64), np.float32)
    d2 = np.zeros((64, 9, 128), np.float32)
    d3 = np.zeros((128, 9, 128), np.float32)
    d4 = np.zeros((128, 9, 2, 128), np.float32)
    d5 = np.zeros((128, 9, 4, 128), np.float32)
    d6 = np.zeros((128, 16, 9, 128), np.float32)
    for i, (dy, dx) in enumerate(DYDX):
        for g in range(8):
            for ci in range(3):
                d1[3 * g + ci, i, 8 * g:8 * g + 8] = w1[8 * g:8 * g + 8, ci, dy, dx]
            for ci in range(8):
                d2[8 * g + ci, i, 16 * g:16 * g + 16] = w2[16 * g:16 * g + 16, ci, dy, dx]
        for oc in range(2):
            for gl in range(4):
                g = 4 * oc + gl
                for ci in range(16):
                    d3[64 * oc + 16 * gl + ci, i, 32 * gl:32 * gl + 32] = \
                        w3[128 * oc + 32 * gl:128 * oc + 32 * gl + 32, ci, dy, dx]
        for oc in range(4):
            b0 = 64 * (oc % 2)
            for gl in range(2):
                for ci in range(32):
                    d4[b0 + 32 * gl + ci, i, oc // 2, 64 * gl:64 * gl + 64] = \
                        w4[128 * oc + 64 * gl:128 * oc + 64 * gl + 64, ci, dy, dx]
        for g in range(8):
            b0 = 64 * (g % 2)
            for ci in range(64):
                d5[b0 + ci, i, g // 2, :] = w5[128 * g:128 * g + 128, ci, dy, dx]
        for oc in range(16):
            g, hf = oc // 2, oc % 2
            for ci in range(128):
                d6[ci, oc, i, :] = w6[256 * g + 128 * hf:256 * g + 128 * hf + 128, ci, dy, dx]
    return d1, d2, d3, d4, d5, d6


def _pack_g(gw, nk):
    d = np.zeros((128, 16, nk, 128), np.float32)
    w2d = gw[:, :, 0, 0]
    for oc in range(16):
        for k in range(nk):
            d[:, oc, k, :] = w2d[128 * oc:128 * oc + 128, 128 * k:128 * k + 128].T
    return d


# ---------------------------------------------------------------- program
def _build():
    nc = bacc.Bacc("TRN2", target_bir_lowering=False, debug=False,
                   enable_asserts=False)
    dt_in = lambda n, s: nc.dram_tensor(n, s, f32, kind="ExternalInput").ap()
    dt_out = lambda n, s: nc.dram_tensor(n, s, f32, kind="ExternalOutput").ap()

    img_d = dt_in("img", [3, H, W])
    sem_d = dt_in("sem", [S, H, W])
    d1_d = dt_in("d1", [24, 9, 64])
    d2_d = dt_in("d2", [64, 9, 128])
    d3_d = dt_in("d3", [128, 9, 128])
    d4_d = dt_in("d4", [128, 9, 2, 128])
    d5_d = dt_in("d5", [128, 9, 4, 128])
    d6_d = dt_in("d6", [128, 16, 9, 128])
    dg1_d = dt_in("dg1", [128, 16, 4, 128])
    dg2_d = dt_in("dg2", [128, 16, 2, 128])
    gb1_d = dt_in("gb1", [128, 16])
    gb2_d = dt_in("gb2", [128, 16])
    cones_d = dt_in("cones", [128, 1])
    crow_d = dt_in("crow", [1, 128])
    cid_d = dt_in("cid", [128, 128])
    ciota_d = dt_in("ciota", [128, 256])
    c256_d = dt_in("c256", [128, 2])

    images_d = dt_out("images", [24, H, W])
    x_d = dt_out("x", [2048, 8, 8])
    x1_d = dt_out("x1", [2048, 18, 18])
    x2_d = dt_out("x2", [2048, 34, 34])

    a1raw_d = nc.dram_tensor("a1raw", [64, 128, 128], f32).ap()

    with tile.TileContext(nc) as tc:
        _emit(nc, tc, locals())
    nc.compile()
    return nc


def _norm_scalars(nc, pool, sum_v, sumsq_v, nelem, P, eps=1e-5):
    """sum_v/sumsq_v: [P, n] views. Returns (mean, rstd6, rstd4, b4) [P,1]."""
    sc = pool.tile([P, 8], f32, name=f"nsc{_norm_scalars.i}", tag="nsc")
    _norm_scalars.i += 1
    mean, ex2, var, sdev = sc[:, 0:1], sc[:, 1:2], sc[:, 2:3], sc[:, 3:4]
    rstd6, rstd4, b4, rstd = sc[:, 4:5], sc[:, 5:6], sc[:, 6:7], sc[:, 7:8]
    if sum_v.shape[1] > 1:
        nc.vector.reduce_sum(mean, sum_v, axis=AX.X)
        nc.vector.reduce_sum(ex2, sumsq_v, axis=AX.X)
        nc.vector.tensor_scalar_mul(mean, mean, 1.0 / nelem)
        nc.vector.tensor_scalar_mul(ex2, ex2, 1.0 / nelem)
    else:
        nc.vector.tensor_scalar_mul(mean, sum_v, 1.0 / nelem)
        nc.vector.tensor_scalar_mul(ex2, sumsq_v, 1.0 / nelem)
    nc.vector.tensor_tensor(var, mean, mean, op=OP.mult)
    nc.vector.tensor_tensor(var, ex2, var, op=OP.subtract)
    nc.vector.tensor_scalar_add(var, var, eps)
    nc.scalar.activation(sdev, var, AF.Sqrt)
    nc.vector.reciprocal(rstd, sdev)
    nc.vector.tensor_scalar_mul(rstd6, rstd, 0.6)
    nc.vector.tensor_scalar_mul(rstd4, rstd, 0.4)
    nc.vector.tensor_tensor(b4, mean, rstd4, op=OP.mult)
    nc.vector.tensor_scalar_mul(b4, b4, -1.0)
    return mean, rstd6, rstd4, b4


_norm_scalars.i = 0


def _lrelu_norm(nc, scr_pool, src, dst, mean, rstd6, rstd4, b4, P, free):
    """dst = lrelu((src-mean)*rstd) = 0.6*(src-m)*rstd + 0.4*|(src-m)*rstd|."""
    free = 1
    for d in src.shape[1:]:
        free *= d
    scr = scr_pool.tile([P, free], f32, name=f"lns{_lrelu_norm.i}", tag="lns")
    _lrelu_norm.i += 1
    sv = scr[:]
    if len(src.shape) == 3:
        sv = sv.rearrange("p (a b) -> p a b", a=src.shape[1])
    nc.scalar.activation(sv, src, AF.Abs, bias=b4, scale=rstd4)
    nc.vector.tensor_scalar(dst, src, mean, rstd6, op0=OP.subtract, op1=OP.mult)
    nc.vector.tensor_tensor(dst, dst, sv, op=OP.add)


_lrelu_norm.i = 0


def _lrelu_bias(nc, scr_pool, psum, dst, bias_ap, P):
    """dst = lrelu(psum + bias) = 0.6*(x+b) + 0.4*|x+b|."""
    free = 1
    for d in psum.shape[1:]:
        free *= d
    scr = scr_pool.tile([P, free], f32, name=f"lbs{_lrelu_bias.i}", tag="lns")
    _lrelu_bias.i += 1
    sv = scr[:]
    if len(psum.shape) == 3:
        sv = sv.rearrange("p (a b) -> p a b", a=psum.shape[1])
    b4 = scr_pool.tile([P, 1], f32, name=f"lbb{_lrelu_bias.i}", tag="lbb")
    nc.vector.tensor_scalar_mul(b4[:], bias_ap, 0.4)
    nc.scalar.activation(sv, psum, AF.Abs, bias=b4[:], scale=0.4)
    nc.vector.tensor_scalar(dst, psum, bias_ap, 0.6, op0=OP.add, op1=OP.mult)
    nc.vector.tensor_tensor(dst, dst, sv, op=OP.add)


_lrelu_bias.i = 0


def _emit(nc, tc, t):
    import contextlib
    ctx = contextlib.ExitStack()
    with ctx:
        gp = ctx.enter_context(tc.tile_pool(name="gp", bufs=1))
        tiny = ctx.enter_context(tc.tile_pool(name="tiny", bufs=4))
        scr_pool = ctx.enter_context(tc.tile_pool(name="scr", bufs=2))
        ps_small = ctx.enter_context(tc.tile_pool(name="pss", bufs=4, space="PSUM"))
        ps_main = ctx.enter_context(tc.tile_pool(name="psm", bufs=4, space="PSUM"))

        cones = gp.tile([128, 1], f32)
        nc.sync.dma_start(cones[:], t["cones_d"][:])
        crow = gp.tile([1, 128], f32)
        nc.sync.dma_start(crow[:], t["crow_d"][:])
        cid = gp.tile([128, 128], f32)
        nc.sync.dma_start(cid[:], t["cid_d"][:])
        ciota = gp.tile([128, 256], f32)
        nc.sync.dma_start(ciota[:], t["ciota_d"][:])
        c256 = gp.tile([128, 2], f32)
        nc.sync.dma_start(c256[:], t["c256_d"][:])

        # ---------------- phase T: trans_img ----------------
        img_t = gp.tile([128, 3, 2, 256], f32)
        nc.sync.dma_start(img_t[:], t["img_d"][:].rearrange("c (q p) j -> p c q j", p=128))

        with tc.tile_pool(name="tp", bufs=2) as tp, \
             tc.tile_pool(name="tps", bufs=3) as tps:
            for c in range(8):
                mask = tp.tile([128, 2, 256], f32, name=f"mask{c}", tag="mask")
                nc.sync.dma_start(mask[:], t["sem_d"][c].rearrange("(q p) j -> p q j", p=128))
                seg = tp.tile([128, 3, 2, 256], f32, name=f"seg{c}", tag="seg")
                for ch in range(3):
                    nc.vector.tensor_tensor(seg[:, ch], img_t[:, ch], mask[:], op=OP.mult)

                # column sums (over rows) via ones-matmul; row sums via reduce
                psc = ps_small.tile([1, 256], f32, name=f"psc{c}", tag="psS")
                k = 0
                for ch in range(3):
                    for q in range(2):
                        nc.tensor.matmul(psc[:], cones[:], seg[:, ch, q],
                                         start=(k == 0), stop=(k == 5))
                        k += 1
                rs = tps.tile([128, 3, 2], f32, name=f"rs{c}", tag="rs")
                for ch in range(3):
                    nc.vector.reduce_sum(rs[:, ch], seg[:, ch], axis=AX.X)
                rsum = tps.tile([128, 2], f32, name=f"rsum{c}", tag="rsum")
                nc.vector.tensor_tensor(rsum[:], rs[:, 0], rs[:, 1], op=OP.add)
                nc.vector.tensor_tensor(rsum[:], rsum[:], rs[:, 2], op=OP.add)
                psr0 = ps_small.tile([1, 128], f32, name=f"psr0{c}", tag="psS")
                psr1 = ps_small.tile([1, 128], f32, name=f"psr1{c}", tag="psS")
                nc.tensor.matmul(psr0[:], rsum[:, 0:1], cid[:])
                nc.tensor.matmul(psr1[:], rsum[:, 1:2], cid[:])

                sc4 = tiny.tile([1, 4], f32, name=f"sc4{c}", tag="sc4")
                vrow = tiny.tile([1, 256], f32, name=f"vrow{c}", tag="vrow")
                nc.vector.tensor_copy(vrow[:, 0:128], psr0[:])
                nc.vector.tensor_copy(vrow[:, 128:256], psr1[:])

                for ax, (vsrc, off) in enumerate([(vrow[:], 0), (psc[:], 2)]):
                    eq = tiny.tile([1, 256], f32, name=f"eq{c}_{ax}", tag="eq")
                    nc.vector.tensor_scalar(eq[:], vsrc, 0.0, None, op0=OP.is_equal)
                    tmin = tiny.tile([1, 256], f32, name=f"tm{c}_{ax}", tag="tm")
                    nc.vector.tensor_scalar_mul(tmin[:], eq[:], BIG)
                    nc.vector.tensor_tensor(tmin[:], tmin[:], ciota[0:1, :], op=OP.add)
                    lo = tiny.tile([1, 2], f32, name=f"lo{c}_{ax}", tag="lo")
                    nc.vector.tensor_reduce(lo[:, 0:1], tmin[:], axis=AX.X, op=OP.min)
                    nc.vector.tensor_scalar_mul(tmin[:], eq[:], -BIG)
                    nc.vector.tensor_tensor(tmin[:], tmin[:], ciota[0:1, :], op=OP.add)
                    nc.vector.tensor_reduce(lo[:, 1:2], tmin[:], axis=AX.X, op=OP.max)
                    # sc4[off] = 256*lo ; sc4[off+1] = hi - lo + 1
                    nc.vector.tensor_scalar_mul(sc4[:, off:off + 1], lo[:, 0:1], 256.0)
                    nc.vector.tensor_scalar(sc4[:, off + 1:off + 2], lo[:, 1:2],
                                            lo[:, 0:1], 1.0, op0=OP.subtract, op1=OP.add)

                psb = ps_small.tile([128, 4], f32, name=f"psb{c}", tag="psS")
                nc.tensor.matmul(psb[:], crow[:], sc4[:])
                scb = tiny.tile([128, 4], f32, name=f"scb{c}", tag="scb")
                nc.scalar.copy(scb[:], psb[:])

                sel = tps.tile([128, 4, 256], f32, name=f"sel{c}", tag="sel")  # RT0,RT1,CT0,CT1
                t1 = tps.tile([128, 2, 256], f32, name=f"t1_{c}", tag="t1")
                nc.vector.tensor_scalar(t1[:, 0], ciota[:], scb[:, 1:2], scb[:, 0:1],
                                        op0=OP.mult, op1=OP.add)
                nc.vector.tensor_scalar(t1[:, 1], ciota[:], scb[:, 3:4], scb[:, 2:3],
                                        op0=OP.mult, op1=OP.add)
                t2 = tps.tile([128, 256], f32, name=f"t2_{c}", tag="t2")
                ta = tps.tile([128, 256], f32, name=f"ta_{c}", tag="ta")
                for ax in range(2):
                    for q in range(2):
                        nc.vector.tensor_scalar(t2[:], t1[:, ax], c256[:, q:q + 1],
                                                None, op0=OP.subtract)
                        nc.vector.tensor_scalar(ta[:], t2[:], 0.0, None, op0=OP.is_ge)
                        nc.vector.tensor_scalar(t2[:], t2[:], 256.0, None, op0=OP.is_ge)
                        nc.vector.tensor_tensor(sel[:, 2 * ax + q], ta[:], t2[:],
                                                op=OP.subtract)

                for ch in range(3):
                    wsb = tps.tile([128, 2, 256], f32, name=f"w{c}_{ch}", tag="wsb")
                    for mc in range(2):
                        psw = ps_main.tile([128, 256], f32, name=f"psw{c}{ch}{mc}", tag="psM")
                        for q in range(2):
                            nc.tensor.matmul(psw[:], seg[:, ch, q, 128 * mc:128 * (mc + 1)],
                                             sel[:, q], start=(q == 0), stop=(q == 1))
                        nc.scalar.copy(wsb[:, mc], psw[:])
                    for nch in range(2):
                        psz = ps_main.tile([128, 256], f32, name=f"psz{c}{ch}{nch}", tag="psM")
                        for mc in range(2):
                            nc.tensor.matmul(psz[:], wsb[:, mc, 128 * nch:128 * (nch + 1)],
                                             sel[:, 2 + mc], start=(mc == 0), stop=(mc == 1))
                        rsb = tps.tile([128, 256], f32, name=f"r{c}{ch}{nch}", tag="rsb")
                        nc.scalar.copy(rsb[:], psz[:])
                        nc.sync.dma_start(
                            t["images_d"][3 * c + ch, 128 * nch:128 * (nch + 1), :], rsb[:])

        # ---------------- phase C: convs ----------------
        w1s = gp.tile([24, 9, 64], CDT)
        nc.sync.dma_start(w1s[:], t["d1_d"][:].bitcast(CDT))
        w2s = gp.tile([64, 9, 128], CDT)
        nc.sync.dma_start(w2s[:], t["d2_d"][:].bitcast(CDT))
        w3s = gp.tile([128, 9, 128], CDT)
        nc.sync.dma_start(w3s[:], t["d3_d"][:].bitcast(CDT))
        w4s = gp.tile([128, 9, 2, 128], CDT)
        nc.sync.dma_start(w4s[:], t["d4_d"][:].bitcast(CDT))
        w5s = gp.tile([128, 9, 4, 128], CDT)
        nc.sync.dma_start(w5s[:], t["d5_d"][:].bitcast(CDT))
        gb1s = gp.tile([128, 16], f32)
        nc.sync.dma_start(gb1s[:], t["gb1_d"][:])
        gb2s = gp.tile([128, 16], f32)
        nc.sync.dma_start(gb2s[:], t["gb2_d"][:])

        a2 = gp.tile([128, 66, 66], CDT)
        nc.vector.memset(a2[:].bitcast(f32), 0.0)
        a3 = gp.tile([128, 2, 34, 34], CDT)
        nc.vector.memset(a3[:].bitcast(f32), 0.0)
        a4 = gp.tile([128, 4, 18, 18], CDT)
        nc.vector.memset(a4[:].bitcast(f32), 0.0)
        a5 = gp.tile([128, 8, 10, 10], CDT)
        nc.vector.memset(a5[:].bitcast(f32), 0.0)
        xr = gp.tile([128, 16, 64], f32)
        stats = gp.tile([128, 2, 64], f32)  # [:, 0]=sums, [:, 1]=sumsq, col-major per layer
        sq = scr_pool.tile([128, 512], f32, name="sqs", tag="sqs")

        with tc.tile_pool(name="strip", bufs=2) as strip_pool, \
             tc.tile_pool(name="bounce", bufs=3) as bounce_pool, \
             tc.tile_pool(name="wstream", bufs=3) as wpool:

            # conv1: images(DRAM) -> a1raw(DRAM), stats in stats[:, :, 0:32]
            for s in range(16):
                o0 = 8 * s
                st = strip_pool.tile([24, 17, 258], CDT, name=f"st1_{s}", tag="strip")
                nc.vector.memset(st[:, :, 0:1].bitcast(f32), 0.0)
                nc.vector.memset(st[:, :, 257:258].bitcast(f32), 0.0)
                lo = max(0, 2 * o0 - 1)
                hi = min(255, 2 * o0 + 15)
                off = lo - (2 * o0 - 1)
                if s == 0:
                    nc.vector.memset(st[:, 0:1, :].bitcast(f32), 0.0)
                nc.sync.dma_start(st[:, off:off + hi - lo + 1, 1:257],
                                  t["images_d"][:, lo:hi + 1, :].bitcast(CDT))
                for p in range(2):
                    ps = ps_main.tile([64, 4, 128], f32, name=f"c1p{s}{p}", tag="psM")
                    for i, (dy, dx) in enumerate(DYDX):
                        rhs = st[:, 8 * p + dy:8 * p + dy + 7:2, dx:dx + 256:2]
                        nc.tensor.matmul(ps[:], w1s[:, i, :], rhs,
                                         start=(i == 0), stop=(i == 8))
                    bt = bounce_pool.tile([64, 4, 128], f32, name=f"b1{s}{p}", tag="b1")
                    pt = 2 * s + p
                    nc.scalar.activation(bt[:], ps[:], AF.Copy,
                                         accum_out=stats[0:64, 0, pt:pt + 1])
                    nc.scalar.activation(sq[0:64, :], ps[:].rearrange("p a b -> p (a b)"),
                                         AF.Square, accum_out=stats[0:64, 1, pt:pt + 1])
                    nc.sync.dma_start(t["a1raw_d"][:, o0 + 4 * p:o0 + 4 * p + 4, :], bt[:])

            m1, r61, r41, b41 = _norm_scalars(nc, tiny, stats[0:64, 0, 0:32],
                                              stats[0:64, 1, 0:32], 16384.0, 64)

            # conv2: a1raw strips (normalize on load) -> a2 interior
            for s in range(8):
                o0 = 8 * s
                st = strip_pool.tile([64, 17, 130], CDT, name=f"st2_{s}", tag="strip")
                nc.vector.memset(st[:, :, 0:1].bitcast(f32), 0.0)
                nc.vector.memset(st[:, :, 129:130].bitcast(f32), 0.0)
                lo = max(0, 2 * o0 - 1)
                hi = min(127, 2 * o0 + 15)
                off = lo - (2 * o0 - 1)
                if s == 0:
                    nc.vector.memset(st[:, 0:1, :].bitcast(f32), 0.0)
                nc.sync.dma_start(st[:, off:off + hi - lo + 1, 1:129],
                                  t["a1raw_d"][:, lo:hi + 1, :].bitcast(CDT))
                v = st[:, off:off + hi - lo + 1, 1:129]
                _lrelu_norm(nc, scr_pool, v, v, m1, r61, r41, b41, 64, 17 * 128)
                ps = ps_main.tile([128, 8, 64], f32, name=f"c2p{s}", tag="psM")
                for i, (dy, dx) in enumerate(DYDX):
                    rhs = st[:, dy:dy + 15:2, dx:dx + 128:2]
                    nc.tensor.matmul(ps[:], w2s[:, i, :], rhs,
                                     start=(i == 0), stop=(i == 8))
                nc.scalar.activation(a2[:, 1 + o0:1 + o0 + 8, 1:65],
                                     ps[:], AF.Copy,
                                     accum_out=stats[:, 0, 32 + s:33 + s])
                nc.scalar.activation(sq[:, :], ps[:].rearrange("p a b -> p (a b)"),
                                     AF.Square, accum_out=stats[:, 1, 32 + s:33 + s])

            m2, r62, r42, b42 = _norm_scalars(nc, tiny, stats[:, 0, 32:40],
                                              stats[:, 1, 32:40], 4096.0, 128)
            for hh in range(4):
                v = a2[:, 1 + 16 * hh:17 + 16 * hh, 1:65]
                _lrelu_norm(nc, scr_pool, v, v, m2, r62, r42, b42, 128, 16 * 64)

            # conv3: a2 -> a3
            for oc in range(2):
                for p in range(2):
                    ps = ps_main.tile([128, 16, 32], f32, name=f"c3p{oc}{p}", tag="psM")
                    for i, (dy, dx) in enumerate(DYDX):
                        rhs = a2[64 * oc:64 * oc + 64, 32 * p + dy:32 * p + dy + 32:2,
                                 dx:dx + 64:2]
                        nc.tensor.matmul(ps[:], w3s[64 * oc:64 * oc + 64, i, :], rhs,
                                         start=(i == 0), stop=(i == 8))
                    pt = 40 + 2 * oc + p
                    nc.scalar.activation(a3[:, oc, 1 + 16 * p:1 + 16 * p + 16, 1:33],
                                         ps[:], AF.Copy, accum_out=stats[:, 0, pt:pt + 1])
                    nc.scalar.activation(sq[:, :], ps[:].rearrange("p a b -> p (a b)"),
                                         AF.Square, accum_out=stats[:, 1, pt:pt + 1])
            for oc in range(2):
                m, r6, r4, b4 = _norm_scalars(nc, tiny, stats[:, 0, 40 + 2 * oc:42 + 2 * oc],
                                              stats[:, 1, 40 + 2 * oc:42 + 2 * oc], 1024.0, 128)
                v = a3[:, oc, 1:33, 1:33]
                _lrelu_norm(nc, scr_pool, v, v, m, r6, r4, b4, 128, 32 * 32)

            # conv4: a3 -> a4
            for oc in range(4):
                b0 = 64 * (oc % 2)
                ps = ps_main.tile([128, 16, 16], f32, name=f"c4p{oc}", tag="psM")
                for i, (dy, dx) in enumerate(DYDX):
                    rhs = a3[b0:b0 + 64, oc // 2, dy:dy + 32:2, dx:dx + 32:2]
                    nc.tensor.matmul(ps[:], w4s[b0:b0 + 64, i, oc // 2, :], rhs,
                                     start=(i == 0), stop=(i == 8))
                pt = 44 + oc
                nc.scalar.activation(a4[:, oc, 1:17, 1:17], ps[:], AF.Copy,
                                     accum_out=stats[:, 0, pt:pt + 1])
                nc.scalar.activation(sq[:, 0:256], ps[:].rearrange("p a b -> p (a b)"),
                                     AF.Square, accum_out=stats[:, 1, pt:pt + 1])
            for oc in range(4):
                m, r6, r4, b4 = _norm_scalars(nc, tiny, stats[:, 0, 44 + oc:45 + oc],
                                              stats[:, 1, 44 + oc:45 + oc], 256.0, 128)
                v = a4[:, oc, 1:17, 1:17]
                _lrelu_norm(nc, scr_pool, v, v, m, r6, r4, b4, 128, 16 * 16)

            # conv5: a4 -> a5
            for oc in range(8):
                b0 = 64 * (oc % 2)
                ps = ps_main.tile([128, 8, 8], f32, name=f"c5p{oc}", tag="psM")
                for i, (dy, dx) in enumerate(DYDX):
                    rhs = a4[b0:b0 + 64, oc // 2, dy:dy + 16:2, dx:dx + 16:2]
                    nc.tensor.matmul(ps[:], w5s[b0:b0 + 64, i, oc // 2, :], rhs,
                                     start=(i == 0), stop=(i == 8))
                pt = 48 + oc
                nc.scalar.activation(a5[:, oc, 1:9, 1:9], ps[:], AF.Copy,
                                     accum_out=stats[:, 0, pt:pt + 1])
                nc.scalar.activation(sq[:, 0:64], ps[:].rearrange("p a b -> p (a b)"),
                                     AF.Square, accum_out=stats[:, 1, pt:pt + 1])
            for oc in range(8):
                m, r6, r4, b4 = _norm_scalars(nc, tiny, stats[:, 0, 48 + oc:49 + oc],
                                              stats[:, 1, 48 + oc:49 + oc], 64.0, 128)
                v = a5[:, oc, 1:9, 1:9]
                _lrelu_norm(nc, scr_pool, v, v, m, r6, r4, b4, 128, 64)

            # conv6 (stride 1): a5 -> xr (raw fp32)
            for oc in range(16):
                wt = wpool.tile([128, 9, 128], CDT, name=f"w6t{oc}", tag="w6t")
                nc.sync.dma_start(wt[:], t["d6_d"][:, oc].bitcast(CDT))
                ps = ps_main.tile([128, 8, 8], f32, name=f"c6p{oc}", tag="psM")
                for i, (dy, dx) in enumerate(DYDX):
                    rhs = a5[:, oc // 2, dy:dy + 8, dx:dx + 8]
                    nc.tensor.matmul(ps[:], wt[:, i, :], rhs,
                                     start=(i == 0), stop=(i == 8))
                nc.scalar.activation(xr[:, oc], ps[:].rearrange("p a b -> p (a b)"),
                                     AF.Copy, accum_out=stats[:, 0, oc:oc + 1])
                nc.scalar.activation(sq[:, 0:64], ps[:].rearrange("p a b -> p (a b)"),
                                     AF.Square, accum_out=stats[:, 1, oc:oc + 1])
            for oc in range(16):
                m, r6, r4, b4 = _norm_scalars(nc, tiny, stats[:, 0, oc:oc + 1],
                                              stats[:, 1, oc:oc + 1], 64.0, 128)
                xb = bounce_pool.tile([128, 64], f32, name=f"xb{oc}", tag="xb")
                _lrelu_norm(nc, scr_pool, xr[:, oc], xb[:], m, r6, r4, b4, 128, 64)
                nc.sync.dma_start(t["x_d"][128 * oc:128 * (oc + 1)].rearrange(
                    "c a b -> c (a b)"), xb[:])

            # border value lrelu(bias) for padded 1x1 conv outputs
            bv = gp.tile([128, 2, 16], f32)
            for bi, gbs in enumerate((gb1s, gb2s)):
                nc.scalar.activation(bv[:, bi], gbs[:], AF.Abs, scale=0.4)
                nc.vector.tensor_scalar(sq[:, 0:16], gbs[:], 0.6, None, op0=OP.mult)
                nc.vector.tensor_tensor(bv[:, bi], bv[:, bi], sq[:, 0:16], op=OP.add)

            # g1: 1x1 on a4 interior -> x1 (18x18 padded output)
            for oc in range(16):
                gt = wpool.tile([128, 4, 128], CDT, name=f"g1t{oc}", tag="g1t")
                nc.sync.dma_start(gt[:], t["dg1_d"][:, oc].bitcast(CDT))
                ps = ps_main.tile([128, 16, 16], f32, name=f"g1p{oc}", tag="psM")
                for k in range(4):
                    nc.tensor.matmul(ps[:], gt[:, k, :], a4[:, k, 1:17, 1:17],
                                     start=(k == 0), stop=(k == 3))
                xo = bounce_pool.tile([128, 18, 18], f32, name=f"x1b{oc}", tag="x1b")
                nc.vector.memset(xo[:], 0.0)
                nc.vector.tensor_scalar(xo[:], xo[:], bv[:, 0, oc:oc + 1], None, op0=OP.add)
                _lrelu_bias(nc, scr_pool, ps[:], xo[:, 1:17, 1:17], gb1s[:, oc:oc + 1], 128)
                nc.sync.dma_start(t["x1_d"][128 * oc:128 * (oc + 1)].rearrange(
                    "c a b -> c (a b)"), xo[:].rearrange("p a b -> p (a b)"))

            # g2: 1x1 on a3 interior -> x2
            for oc in range(16):
                gt = wpool.tile([128, 2, 128], CDT, name=f"g2t{oc}", tag="g2t")
                nc.sync.dma_start(gt[:], t["dg2_d"][:, oc].bitcast(CDT))
                xo = bounce_pool.tile([128, 34, 34], f32, name=f"x2b{oc}", tag="x2b")
                nc.vector.memset(xo[:], 0.0)
                nc.vector.tensor_scalar(xo[:], xo[:], bv[:, 1, oc:oc + 1], None, op0=OP.add)
                for nh in range(2):
                    ps = ps_main.tile([128, 16, 32], f32, name=f"g2p{oc}{nh}", tag="psM")
                    for k in range(2):
                        nc.tensor.matmul(ps[:], gt[:, k, :],
                                         a3[:, k, 1 + 16 * nh:17 + 16 * nh, 1:33],
                                         start=(k == 0), stop=(k == 1))
                    _lrelu_bias(nc, scr_pool, ps[:], xo[:, 1 + 16 * nh:17 + 16 * nh, 1:33],
                                gb2s[:, oc:oc + 1], 128)
                nc.sync.dma_start(t["x2_d"][128 * oc:128 * (oc + 1)].rearrange(
                    "c a b -> c (a b)"), xo[:].rearrange("p a b -> p (a b)"))


# ---------------------------------------------------------------- entry
def kernel(real_image, input_semantic, w1, w2, w3, w4, w5, w6, gw1, gb1, gw2, gb2):
    global LAST_EXEC_NS, LAST_RESULTS
    real_image = np.asarray(real_image, np.float32)
    input_semantic = np.asarray(input_semantic, np.float32)

    if "nc" not in _CACHE:
        _CACHE["nc"] = _build()
    nc = _CACHE["nc"]

    d1, d2, d3, d4, d5, d6 = _pack_weights([np.asarray(w, np.float32)
                                            for w in (w1, w2, w3, w4, w5, w6)])
    dg1 = _pack_g(np.asarray(gw1, np.float32), 4)
    dg2 = _pack_g(np.asarray(gw2, np.float32), 2)
    dgb1 = np.asarray(gb1, np.float32).reshape(16, 128).T.copy()
    dgb2 = np.asarray(gb2, np.float32).reshape(16, 128).T.copy()

    cones = np.ones((128, 1), np.float32)
    crow = np.ones((1, 128), np.float32)
    cid = np.eye(128, dtype=np.float32)
    ciota = np.broadcast_to(np.arange(256, dtype=np.float32), (128, 256)).copy()
    c256 = np.stack([np.arange(128, dtype=np.float32) * 256,
                     (np.arange(128, dtype=np.float32) + 128) * 256], 1).copy()

    shared = dict(d1=d1, d2=d2, d3=d3, d4=d4, d5=d5, d6=d6, dg1=dg1, dg2=dg2,
                  gb1=dgb1, gb2=dgb2, cones=cones, crow=crow, cid=cid,
                  ciota=ciota, c256=c256)
    in_maps = [dict(img=real_image[b], sem=input_semantic[b], **shared)
               for b in range(B)]

    trace = bool(os.environ.get("BASS_TRACE_RUN"))
    if trace:
        import ntff_shim
        ntff_shim.install()
    res = run_bass_kernel_spmd(nc, in_maps, core_ids=list(range(8)), trace=trace)
    LAST_EXEC_NS = res.exec_time_ns
    LAST_RESULTS = res
    x = np.stack([res.results[b]["x"] for b in range(B)])
    x1 = np.stack([res.results[b]["x1"] for b in range(B)])
    x2 = np.stack([res.results[b]["x2"] for b in range(B)])
    images = np.stack([res.results[b]["images"] for b in range(B)])
    return x, x1, x2, images


# revision 9
# speedup vs baseline: 1.0482x; 1.0482x over previous
"""Trainium2 Bass kernel for nn_Conv_ImgEncoder (dense_cnn).

Sharding: data-parallel over batch (8 samples -> 8 cores). Each core runs the
full per-sample pipeline: trans_img (mask crop+nearest resize via exact fp32
one-hot matmul gathers) followed by the grouped-conv stack (float32r matmuls,
fp32 PSUM accumulation) with fused InstanceNorm+LeakyReLU epilogues.
"""
import os
import sys
import numpy as np

for _p in ('/opt/trn_rl_repo',):
    if _p not in sys.path:
        sys.path.insert(0, _p)

import concourse.bass as bass
import concourse.tile as tile
from concourse import bacc, mybir
from concourse.bass_utils import run_bass_kernel_spmd

f32 = mybir.dt.float32
F32R = mybir.dt.float32r
CDT = F32R if os.environ.get("BASS_CONV_DTYPE", "f32r") == "f32r" else f32
AF = mybir.ActivationFunctionType
OP = mybir.AluOpType
AX = mybir.AxisListType

B, S, H, W = 8, 8, 256, 256
BIG = float(2 ** 30)
DYDX = [(dy, dx) for dy in range(3) for dx in range(3)]

LAST_EXEC_NS = None
LAST_RESULTS = None
_CACHE = {}


# ---------------------------------------------------------------- host prep
def _sn_np(w, seed):
    o = w.shape[0]
    m = w.reshape(o, -1).astype(np.float32)
    u = np.random.RandomState(seed).randn(o).astype(np.float32)
    u = u / (np.linalg.norm(u) + 1e-12)
    v = m.T @ u
    v = v / (np.linalg.norm(v) + 1e-12)
    u2 = m @ v
    u2 = u2 / (np.linalg.norm(u2) + 1e-12)
    sigma = u2 @ (m @ v)
    return (w / sigma).astype(np.float32)


def _pack_weights(ws):
    w1, w2, w3, w4, w5, w6 = [_sn_np(w, i + 1) for i, w in enumerate(ws)]
    d1 = np.zeros((24, 9, 64), np.float32)
    d2 = np.zeros((64, 9, 128), np.float32)
    d3 = np.zeros((128, 9, 128), np.float32)
    d4 = np.zeros((128, 9, 2, 128), np.float32)
    d5 = np.zeros((128, 9, 4, 128), np.float32)
    d6 = np.zeros((128, 16, 9, 128), np.float32)
    for i, (dy, dx) in enumerate(DYDX):
        for g in range(8):
            for ci in range(3):
                d1[3 * g + ci, i, 8 * g:8 * g + 8] = w1[8 * g:8 * g + 8, ci, dy, dx]
            for ci in range(8):
                d2[8 * g + ci, i, 16 * g:16 * g + 16] = w2[16 * g:16 * g + 16, ci, dy, dx]
        for oc in range(2):
            for gl in range(4):
                g = 4 * oc + gl
                for ci in range(16):
                    d3[64 * oc + 16 * gl + ci, i, 32 * gl:32 * gl + 32] = \
                        w3[128 * oc + 32 * gl:128 * oc + 32 * gl + 32, ci, dy, dx]
        for oc in range(4):
            b0 = 64 * (oc % 2)
            for gl in range(2):
                for ci in range(32):
                    d4[b0 + 32 * gl + ci, i, oc // 2, 64 * gl:64 * gl + 64] = \
                        w4[128 * oc + 64 * gl:128 * oc + 64 * gl + 64, ci, dy, dx]
        for g in range(8):
            b0 = 64 * (g % 2)
            for ci in range(64):
                d5[b0 + ci, i, g // 2, :] = w5[128 * g:128 * g + 128, ci, dy, dx]
        for oc in range(16):
            g, hf = oc // 2, oc % 2
            for ci in range(128):
                d6[ci, oc, i, :] = w6[256 * g + 128 * hf:256 * g + 128 * hf + 128, ci, dy, dx]
    return d1, d2, d3, d4, d5, d6


def _pack_g(gw, nk):
    d = np.zeros((128, 16, nk, 128), np.float32)
    w2d = gw[:, :, 0, 0]
    for oc in range(16):
        for k in range(nk):
            d[:, oc, k, :] = w2d[128 * oc:128 * oc + 128, 128 * k:128 * k + 128].T
    return d


# ---------------------------------------------------------------- program
def _build():
    nc = bacc.Bacc("TRN2", target_bir_lowering=False, debug=False,
                   enable_asserts=False)
    dt_in = lambda n, s: nc.dram_tensor(n, s, f32, kind="ExternalInput").ap()
    dt_out = lambda n, s: nc.dram_tensor(n, s, f32, kind="ExternalOutput").ap()

    img_d = dt_in("img", [3, H, W])
    sem_d = dt_in("sem", [S, H, W])
    d1_d = dt_in("d1", [24, 9, 64])
    d2_d = dt_in("d2", [64, 9, 128])
    d3_d = dt_in("d3", [128, 9, 128])
    d4_d = dt_in("d4", [128, 9, 2, 128])
    d5_d = dt_in("d5", [128, 9, 4, 128])
    d6_d = dt_in("d6", [128, 16, 9, 128])
    dg1_d = dt_in("dg1", [128, 16, 4, 128])
    dg2_d = dt_in("dg2", [128, 16, 2, 128])
    gb1_d = dt_in("gb1", [128, 16])
    gb2_d = dt_in("gb2", [128, 16])
    cones_d = dt_in("cones", [128, 1])
    crow_d = dt_in("crow", [1, 128])
    cid_d = dt_in("cid", [128, 128])
    ciota_d = dt_in("ciota", [128, 256])
    c256_d = dt_in("c256", [128, 2])

    images_d = dt_out("images", [24, H, W])
    x_d = dt_out("x", [2048, 8, 8])
    x1_d = dt_out("x1", [2048, 18, 18])
    x2_d = dt_out("x2", [2048, 34, 34])

    a1raw_d = nc.dram_tensor("a1raw", [64, 128, 128], f32).ap()

    with tile.TileContext(nc) as tc:
        _emit(nc, tc, locals())
    nc.compile()
    return nc


def _norm_scalars(nc, pool, sum_v, sumsq_v, nelem, P, eps=1e-5):
    """sum_v/sumsq_v: [P, n] views. Returns (mean, rstd6, rstd4, b4) [P,1]."""
    sc = pool.tile([P, 8], f32, name=f"nsc{_norm_scalars.i}", tag="nsc")
    _norm_scalars.i += 1
    mean, ex2, var, sdev = sc[:, 0:1], sc[:, 1:2], sc[:, 2:3], sc[:, 3:4]
    rstd6, rstd4, b4, rstd = sc[:, 4:5], sc[:, 5:6], sc[:, 6:7], sc[:, 7:8]
    if sum_v.shape[1] > 1:
        nc.vector.reduce_sum(mean, sum_v, axis=AX.X)
        nc.vector.reduce_sum(ex2, sumsq_v, axis=AX.X)
        nc.vector.tensor_scalar_mul(mean, mean, 1.0 / nelem)
        nc.vector.tensor_scalar_mul(ex2, ex2, 1.0 / nelem)
    else:
        nc.vector.tensor_scalar_mul(mean, sum_v, 1.0 / nelem)
        nc.vector.tensor_scalar_mul(ex2, sumsq_v, 1.0 / nelem)
    nc.vector.tensor_tensor(var, mean, mean, op=OP.mult)
    nc.vector.tensor_tensor(var, ex2, var, op=OP.subtract)
    nc.vector.tensor_scalar_add(var, var, eps)
    nc.scalar.activation(sdev, var, AF.Sqrt)
    nc.vector.reciprocal(rstd, sdev)
    nc.vector.tensor_scalar_mul(rstd6, rstd, 0.6)
    nc.vector.tensor_scalar_mul(rstd4, rstd, 0.4)
    nc.vector.tensor_tensor(b4, mean, rstd4, op=OP.mult)
    nc.vector.tensor_scalar_mul(b4, b4, -1.0)
    return mean, rstd6, rstd4, b4


_norm_scalars.i = 0


def _lrelu_norm(nc, scr_pool, src, dst, mean, rstd6, rstd4, b4, P, free):
    """dst = lrelu((src-mean)*rstd) = 0.6*(src-m)*rstd + 0.4*|(src-m)*rstd|."""
    free = 1
    for d in src.shape[1:]:
        free *= d
    scr = scr_pool.tile([P, free], f32, name=f"lns{_lrelu_norm.i}", tag="lns")
    _lrelu_norm.i += 1
    sv = scr[:]
    if len(src.shape) == 3:
        sv = sv.rearrange("p (a b) -> p a b", a=src.shape[1])
    nc.scalar.activation(sv, src, AF.Abs, bias=b4, scale=rstd4)
    nc.vector.tensor_scalar(dst, src, mean, rstd6, op0=OP.subtract, op1=OP.mult)
    nc.vector.tensor_tensor(dst, dst, sv, op=OP.add)


_lrelu_norm.i = 0


def _lrelu_bias(nc, scr_pool, psum, dst, bias_ap, P):
    """dst = lrelu(psum + bias) = 0.6*(x+b) + 0.4*|x+b|."""
    free = 1
    for d in psum.shape[1:]:
        free *= d
    scr = scr_pool.tile([P, free], f32, name=f"lbs{_lrelu_bias.i}", tag="lns")
    _lrelu_bias.i += 1
    sv = scr[:]
    if len(psum.shape) == 3:
        sv = sv.rearrange("p (a b) -> p a b", a=psum.shape[1])
    b4 = scr_pool.tile([P, 1], f32, name=f"lbb{_lrelu_bias.i}", tag="lbb")
    nc.vector.tensor_scalar_mul(b4[:], bias_ap, 0.4)
    nc.scalar.activation(sv, psum, AF.Abs, bias=b4[:], scale=0.4)
    nc.vector.tensor_scalar(dst, psum, bias_ap, 0.6, op0=OP.add, op1=OP.mult)
    nc.vector.tensor_tensor(dst, dst, sv, op=OP.add)


_lrelu_bias.i = 0


def _emit(nc, tc, t):
    import contextlib
    ctx = contextlib.ExitStack()
    with ctx:
        gp = ctx.enter_context(tc.tile_pool(name="gp", bufs=1))
        tiny = ctx.enter_context(tc.tile_pool(name="tiny", bufs=4))
        scr_pool = ctx.enter_context(tc.tile_pool(name="scr", bufs=2))
        ps_small = ctx.enter_context(tc.tile_pool(name="pss", bufs=4, space="PSUM"))
        ps_main = ctx.enter_context(tc.tile_pool(name="psm", bufs=4, space="PSUM"))

        cones = gp.tile([128, 1], f32)
        nc.sync.dma_start(cones[:], t["cones_d"][:])
        crow = gp.tile([1, 128], f32)
        nc.sync.dma_start(crow[:], t["crow_d"][:])
        cid = gp.tile([128, 128], f32)
        nc.sync.dma_start(cid[:], t["cid_d"][:])
        ciota = gp.tile([128, 256], f32)
        nc.sync.dma_start(ciota[:], t["ciota_d"][:])
        c256 = gp.tile([128, 2], f32)
        nc.sync.dma_start(c256[:], t["c256_d"][:])

        # ---------------- phase T: trans_img ----------------
        img_t = gp.tile([128, 3, 2, 256], f32)
        nc.sync.dma_start(img_t[:], t["img_d"][:].rearrange("c (q p) j -> p c q j", p=128))

        with tc.tile_pool(name="tp", bufs=2) as tp, \
             tc.tile_pool(name="tps", bufs=3) as tps:
            for c in range(8):
                mask = tp.tile([128, 2, 256], f32, name=f"mask{c}", tag="mask")
                nc.sync.dma_start(mask[:], t["sem_d"][c].rearrange("(q p) j -> p q j", p=128))
                seg = tp.tile([128, 3, 2, 256], f32, name=f"seg{c}", tag="seg")
                for ch in range(3):
                    nc.vector.tensor_tensor(seg[:, ch], img_t[:, ch], mask[:], op=OP.mult)

                # column sums (over rows) via ones-matmul; row sums via reduce
                psc = ps_small.tile([1, 256], f32, name=f"psc{c}", tag="psS")
                k = 0
                for ch in range(3):
                    for q in range(2):
                        nc.tensor.matmul(psc[:], cones[:], seg[:, ch, q],
                                         start=(k == 0), stop=(k == 5))
                        k += 1
                rs = tps.tile([128, 3, 2], f32, name=f"rs{c}", tag="rs")
                for ch in range(3):
                    nc.vector.reduce_sum(rs[:, ch], seg[:, ch], axis=AX.X)
                rsum = tps.tile([128, 2], f32, name=f"rsum{c}", tag="rsum")
                nc.vector.tensor_tensor(rsum[:], rs[:, 0], rs[:, 1], op=OP.add)
                nc.vector.tensor_tensor(rsum[:], rsum[:], rs[:, 2], op=OP.add)
                psr0 = ps_small.tile([1, 128], f32, name=f"psr0{c}", tag="psS")
                psr1 = ps_small.tile([1, 128], f32, name=f"psr1{c}", tag="psS")
                nc.tensor.matmul(psr0[:], rsum[:, 0:1], cid[:])
                nc.tensor.matmul(psr1[:], rsum[:, 1:2], cid[:])

                sc4 = tiny.tile([1, 4], f32, name=f"sc4{c}", tag="sc4")
                vrow = tiny.tile([1, 256], f32, name=f"vrow{c}", tag="vrow")
                nc.vector.tensor_copy(vrow[:, 0:128], psr0[:])
                nc.vector.tensor_copy(vrow[:, 128:256], psr1[:])

                for ax, (vsrc, off) in enumerate([(vrow[:], 0), (psc[:], 2)]):
                    eq = tiny.tile([1, 256], f32, name=f"eq{c}_{ax}", tag="eq")
                    nc.vector.tensor_scalar(eq[:], vsrc, 0.0, None, op0=OP.is_equal)
                    tmin = tiny.tile([1, 256], f32, name=f"tm{c}_{ax}", tag="tm")
                    nc.vector.tensor_scalar_mul(tmin[:], eq[:], BIG)
                    nc.vector.tensor_tensor(tmin[:], tmin[:], ciota[0:1, :], op=OP.add)
                    lo = tiny.tile([1, 2], f32, name=f"lo{c}_{ax}", tag="lo")
                    nc.vector.tensor_reduce(lo[:, 0:1], tmin[:], axis=AX.X, op=OP.min)
                    nc.vector.tensor_scalar_mul(tmin[:], eq[:], -BIG)
                    nc.vector.tensor_tensor(tmin[:], tmin[:], ciota[0:1, :], op=OP.add)
                    nc.vector.tensor_reduce(lo[:, 1:2], tmin[:], axis=AX.X, op=OP.max)
                    # sc4[off] = 256*lo ; sc4[off+1] = hi - lo + 1
                    nc.vector.tensor_scalar_mul(sc4[:, off:off + 1], lo[:, 0:1], 256.0)
                    nc.vector.tensor_scalar(sc4[:, off + 1:off + 2], lo[:, 1:2],
                                            lo[:, 0:1], 1.0, op0=OP.subtract, op1=OP.add)

                psb = ps_small.tile([128, 4], f32, name=f"psb{c}", tag="psS")
                nc.tensor.matmul(psb[:], crow[:], sc4[:])
                scb = tiny.tile([128, 4], f32, name=f"scb{c}", tag="scb")
                nc.scalar.copy(scb[:], psb[:])

                sel = tps.tile([128, 4, 256], f32, name=f"sel{c}", tag="sel")  # RT0,RT1,CT0,CT1
                t1 = tps.tile([128, 2, 256], f32, name=f"t1_{c}", tag="t1")
                nc.vector.tensor_scalar(t1[:, 0], ciota[:], scb[:, 1:2], scb[:, 0:1],
                                        op0=OP.mult, op1=OP.add)
                nc.vector.tensor_scalar(t1[:, 1], ciota[:], scb[:, 3:4], scb[:, 2:3],
                                        op0=OP.mult, op1=OP.add)
                t2 = tps.tile([128, 256], f32, name=f"t2_{c}", tag="t2")
                ta = tps.tile([128, 256], f32, name=f"ta_{c}", tag="ta")
                for ax in range(2):
                    for q in range(2):
                        nc.vector.tensor_scalar(t2[:], t1[:, ax], c256[:, q:q + 1],
                                                None, op0=OP.subtract)
                        nc.vector.tensor_scalar(ta[:], t2[:], 0.0, None, op0=OP.is_ge)
                        nc.vector.tensor_scalar(t2[:], t2[:], 256.0, None, op0=OP.is_ge)
                        nc.vector.tensor_tensor(sel[:, 2 * ax + q], ta[:], t2[:],
                                                op=OP.subtract)

                for ch in range(3):
                    wsb = tps.tile([128, 2, 256], f32, name=f"w{c}_{ch}", tag="wsb")
                    for mc in range(2):
                        psw = ps_main.tile([128, 256], f32, name=f"psw{c}{ch}{mc}", tag="psM")
                        for q in range(2):
                            nc.tensor.matmul(psw[:], seg[:, ch, q, 128 * mc:128 * (mc + 1)],
                                             sel[:, q], start=(q == 0), stop=(q == 1))
                        nc.scalar.copy(wsb[:, mc], psw[:])
                    for nch in range(2):
                        psz = ps_main.tile([128, 256], f32, name=f"psz{c}{ch}{nch}", tag="psM")
                        for mc in range(2):
                            nc.tensor.matmul(psz[:], wsb[:, mc, 128 * nch:128 * (nch + 1)],
                                             sel[:, 2 + mc], start=(mc == 0), stop=(mc == 1))
                        rsb = tps.tile([128, 256], f32, name=f"r{c}{ch}{nch}", tag="rsb")
                        nc.scalar.copy(rsb[:], psz[:])
                        nc.sync.dma_start(
                            t["images_d"][3 * c + ch, 128 * nch:128 * (nch + 1), :], rsb[:])

        # ---------------- phase C: convs ----------------
        w1s = gp.tile([24, 9, 64], CDT)
        nc.sync.dma_start(w1s[:], t["d1_d"][:].bitcast(CDT))
        w2s = gp.tile([64, 9, 128], CDT)
        nc.sync.dma_start(w2s[:], t["d2_d"][:].bitcast(CDT))
        w3s = gp.tile([128, 9, 128], CDT)
        nc.sync.dma_start(w3s[:], t["d3_d"][:].bitcast(CDT))
        w4s = gp.tile([128, 9, 2, 128], CDT)
        nc.sync.dma_start(w4s[:], t["d4_d"][:].bitcast(CDT))
        w5s = gp.tile([128, 9, 4, 128], CDT)
        nc.sync.dma_start(w5s[:], t["d5_d"][:].bitcast(CDT))
        gb1s = gp.tile([128, 16], f32)
        nc.sync.dma_start(gb1s[:], t["gb1_d"][:])
        gb2s = gp.tile([128, 16], f32)
        nc.sync.dma_start(gb2s[:], t["gb2_d"][:])

        a2 = gp.tile([128, 66, 66], CDT)
        nc.vector.memset(a2[:].bitcast(f32), 0.0)
        a3 = gp.tile([128, 2, 34, 34], CDT)
        nc.vector.memset(a3[:].bitcast(f32), 0.0)
        a4 = gp.tile([128, 4, 18, 18], CDT)
        nc.vector.memset(a4[:].bitcast(f32), 0.0)
        a5 = gp.tile([128, 8, 10, 10], CDT)
        nc.vector.memset(a5[:].bitcast(f32), 0.0)
        xr = gp.tile([128, 16, 64], f32)
        stats = gp.tile([128, 2, 64], f32)  # [:, 0]=sums, [:, 1]=sumsq, col-major per layer
        sq = scr_pool.tile([128, 512], f32, name="sqs", tag="sqs")

        with tc.tile_pool(name="strip", bufs=2) as strip_pool, \
             tc.tile_pool(name="bounce", bufs=3) as bounce_pool, \
             tc.tile_pool(name="wstream", bufs=4) as wpool:

            # conv1: images(DRAM) -> a1raw(DRAM), stats in stats[:, :, 0:32]
            for s in range(16):
                o0 = 8 * s
                st = strip_pool.tile([24, 17, 258], CDT, name=f"st1_{s}", tag="strip")
                nc.vector.memset(st[:, :, 0:1].bitcast(f32), 0.0)
                nc.vector.memset(st[:, :, 257:258].bitcast(f32), 0.0)
                lo = max(0, 2 * o0 - 1)
                hi = min(255, 2 * o0 + 15)
                off = lo - (2 * o0 - 1)
                if s == 0:
                    nc.vector.memset(st[:, 0:1, :].bitcast(f32), 0.0)
                nc.sync.dma_start(st[:, off:off + hi - lo + 1, 1:257],
                                  t["images_d"][:, lo:hi + 1, :].bitcast(CDT))
                for p in range(2):
                    ps = ps_main.tile([64, 4, 128], f32, name=f"c1p{s}{p}", tag="psM")
                    for i, (dy, dx) in enumerate(DYDX):
                        rhs = st[:, 8 * p + dy:8 * p + dy + 7:2, dx:dx + 256:2]
                        nc.tensor.matmul(ps[:], w1s[:, i, :], rhs,
                                         start=(i == 0), stop=(i == 8))
                    bt = bounce_pool.tile([64, 4, 128], f32, name=f"b1{s}{p}", tag="b1")
                    pt = 2 * s + p
                    nc.scalar.activation(bt[:], ps[:], AF.Copy,
                                         accum_out=stats[0:64, 0, pt:pt + 1])
                    nc.scalar.activation(sq[0:64, :], ps[:].rearrange("p a b -> p (a b)"),
                                         AF.Square, accum_out=stats[0:64, 1, pt:pt + 1])
                    nc.sync.dma_start(t["a1raw_d"][:, o0 + 4 * p:o0 + 4 * p + 4, :], bt[:])

            m1, r61, r41, b41 = _norm_scalars(nc, tiny, stats[0:64, 0, 0:32],
                                              stats[0:64, 1, 0:32], 16384.0, 64)

            # conv2: a1raw strips (normalize on load) -> a2 interior
            for s in range(8):
                o0 = 8 * s
                st = strip_pool.tile([64, 17, 130], CDT, name=f"st2_{s}", tag="strip")
                nc.vector.memset(st[:, :, 0:1].bitcast(f32), 0.0)
                nc.vector.memset(st[:, :, 129:130].bitcast(f32), 0.0)
                lo = max(0, 2 * o0 - 1)
                hi = min(127, 2 * o0 + 15)
                off = lo - (2 * o0 - 1)
                if s == 0:
                    nc.vector.memset(st[:, 0:1, :].bitcast(f32), 0.0)
                nc.sync.dma_start(st[:, off:off + hi - lo + 1, 1:129],
                                  t["a1raw_d"][:, lo:hi + 1, :].bitcast(CDT))
                v = st[:, off:off + hi - lo + 1, 1:129]
                _lrelu_norm(nc, scr_pool, v, v, m1, r61, r41, b41, 64, 17 * 128)
                ps = ps_main.tile([128, 8, 64], f32, name=f"c2p{s}", tag="psM")
                for i, (dy, dx) in enumerate(DYDX):
                    rhs = st[:, dy:dy + 15:2, dx:dx + 128:2]
                    nc.tensor.matmul(ps[:], w2s[:, i, :], rhs,
                                     start=(i == 0), stop=(i == 8))
                nc.scalar.activation(a2[:, 1 + o0:1 + o0 + 8, 1:65],
                                     ps[:], AF.Copy,
                                     accum_out=stats[:, 0, 32 + s:33 + s])
                nc.scalar.activation(sq[:, :], ps[:].rearrange("p a b -> p (a b)"),
                                     AF.Square, accum_out=stats[:, 1, 32 + s:33 + s])

            m2, r62, r42, b42 = _norm_scalars(nc, tiny, stats[:, 0, 32:40],
                                              stats[:, 1, 32:40], 4096.0, 128)
            for hh in range(4):
                v = a2[:, 1 + 16 * hh:17 + 16 * hh, 1:65]
                _lrelu_norm(nc, scr_pool, v, v, m2, r62, r42, b42, 128, 16 * 64)

            # conv3: a2 -> a3
            for oc in range(2):
                for p in range(2):
                    ps = ps_main.tile([128, 16, 32], f32, name=f"c3p{oc}{p}", tag="psM")
                    for i, (dy, dx) in enumerate(DYDX):
                        rhs = a2[64 * oc:64 * oc + 64, 32 * p + dy:32 * p + dy + 32:2,
                                 dx:dx + 64:2]
                        nc.tensor.matmul(ps[:], w3s[64 * oc:64 * oc + 64, i, :], rhs,
                                         start=(i == 0), stop=(i == 8))
                    pt = 40 + 2 * oc + p
                    nc.scalar.activation(a3[:, oc, 1 + 16 * p:1 + 16 * p + 16, 1:33],
                                         ps[:], AF.Copy, accum_out=stats[:, 0, pt:pt + 1])
                    nc.scalar.activation(sq[:, :], ps[:].rearrange("p a b -> p (a b)"),
                                         AF.Square, accum_out=stats[:, 1, pt:pt + 1])
            for oc in range(2):
                m, r6, r4, b4 = _norm_scalars(nc, tiny, stats[:, 0, 40 + 2 * oc:42 + 2 * oc],
                                              stats[:, 1, 40 + 2 * oc:42 + 2 * oc], 1024.0, 128)
                v = a3[:, oc, 1:33, 1:33]
                _lrelu_norm(nc, scr_pool, v, v, m, r6, r4, b4, 128, 32 * 32)

            # conv4: a3 -> a4
            for oc in range(4):
                b0 = 64 * (oc % 2)
                ps = ps_main.tile([128, 16, 16], f32, name=f"c4p{oc}", tag="psM")
                for i, (dy, dx) in enumerate(DYDX):
                    rhs = a3[b0:b0 + 64, oc // 2, dy:dy + 32:2, dx:dx + 32:2]
                    nc.tensor.matmul(ps[:], w4s[b0:b0 + 64, i, oc // 2, :], rhs,
                                     start=(i == 0), stop=(i == 8))
                pt = 44 + oc
                nc.scalar.activation(a4[:, oc, 1:17, 1:17], ps[:], AF.Copy,
                                     accum_out=stats[:, 0, pt:pt + 1])
                nc.scalar.activation(sq[:, 0:256], ps[:].rearrange("p a b -> p (a b)"),
                                     AF.Square, accum_out=stats[:, 1, pt:pt + 1])
            for oc in range(4):
                m, r6, r4, b4 = _norm_scalars(nc, tiny, stats[:, 0, 44 + oc:45 + oc],
                                              stats[:, 1, 44 + oc:45 + oc], 256.0, 128)
                v = a4[:, oc, 1:17, 1:17]
                _lrelu_norm(nc, scr_pool, v, v, m, r6, r4, b4, 128, 16 * 16)

            # conv5: a4 -> a5
            for oc in range(8):
                b0 = 64 * (oc % 2)
                ps = ps_main.tile([128, 8, 8], f32, name=f"c5p{oc}", tag="psM")
                for i, (dy, dx) in enumerate(DYDX):
                    rhs = a4[b0:b0 + 64, oc // 2, dy:dy + 16:2, dx:dx + 16:2]
                    nc.tensor.matmul(ps[:], w5s[b0:b0 + 64, i, oc // 2, :], rhs,
                                     start=(i == 0), stop=(i == 8))
                pt = 48 + oc
                nc.scalar.activation(a5[:, oc, 1:9, 1:9], ps[:], AF.Copy,
                                     accum_out=stats[:, 0, pt:pt + 1])
                nc.scalar.activation(sq[:, 0:64], ps[:].rearrange("p a b -> p (a b)"),
                                     AF.Square, accum_out=stats[:, 1, pt:pt + 1])
            for oc in range(8):
                m, r6, r4, b4 = _norm_scalars(nc, tiny, stats[:, 0, 48 + oc:49 + oc],
                                              stats[:, 1, 48 + oc:49 + oc], 64.0, 128)
                v = a5[:, oc, 1:9, 1:9]
                _lrelu_norm(nc, scr_pool, v, v, m, r6, r4, b4, 128, 64)

            # conv6 + g1 + g2 interleaved: independent inputs (a5/a4/a3),
            # keeps PE dense while weights stream
            # border value lrelu(bias) for padded 1x1 conv outputs
            bv = gp.tile([128, 2, 16], f32)
            for bi, gbs in enumerate((gb1s, gb2s)):
                nc.scalar.activation(bv[:, bi], gbs[:], AF.Abs, scale=0.4)
                nc.vector.tensor_scalar(sq[:, 0:16], gbs[:], 0.6, None, op0=OP.mult)
                nc.vector.tensor_tensor(bv[:, bi], bv[:, bi], sq[:, 0:16], op=OP.add)

            for oc in range(16):
                # conv6 (stride 1): a5 -> xr raw + stats, then normalize to x
                wt = wpool.tile([128, 9, 128], CDT, name=f"w6t{oc}", tag="w6t")
                nc.sync.dma_start(wt[:], t["d6_d"][:, oc].bitcast(CDT))
                ps = ps_main.tile([128, 8, 8], f32, name=f"c6p{oc}", tag="psM")
                for i, (dy, dx) in enumerate(DYDX):
                    rhs = a5[:, oc // 2, dy:dy + 8, dx:dx + 8]
                    nc.tensor.matmul(ps[:], wt[:, i, :], rhs,
                                     start=(i == 0), stop=(i == 8))
                nc.scalar.activation(xr[:, oc], ps[:].rearrange("p a b -> p (a b)"),
                                     AF.Copy, accum_out=stats[:, 0, oc:oc + 1])
                nc.scalar.activation(sq[:, 0:64], ps[:].rearrange("p a b -> p (a b)"),
                                     AF.Square, accum_out=stats[:, 1, oc:oc + 1])
                m, r6, r4, b4 = _norm_scalars(nc, tiny, stats[:, 0, oc:oc + 1],
                                              stats[:, 1, oc:oc + 1], 64.0, 128)
                xb = bounce_pool.tile([128, 64], f32, name=f"xb{oc}", tag="xb")
                _lrelu_norm(nc, scr_pool, xr[:, oc], xb[:], m, r6, r4, b4, 128, 64)
                nc.sync.dma_start(t["x_d"][128 * oc:128 * (oc + 1)].rearrange(
                    "c a b -> c (a b)"), xb[:])

                # g1: 1x1 on a4 interior -> x1 (18x18 padded output)
                gt = wpool.tile([128, 4, 128], CDT, name=f"g1t{oc}", tag="g1t")
                nc.sync.dma_start(gt[:], t["dg1_d"][:, oc].bitcast(CDT))
                ps = ps_main.tile([128, 16, 16], f32, name=f"g1p{oc}", tag="psM")
                for k in range(4):
                    nc.tensor.matmul(ps[:], gt[:, k, :], a4[:, k, 1:17, 1:17],
                                     start=(k == 0), stop=(k == 3))
                xo = bounce_pool.tile([128, 18, 18], f32, name=f"x1b{oc}", tag="x1b")
                nc.vector.memset(xo[:], 0.0)
                nc.vector.tensor_scalar(xo[:], xo[:], bv[:, 0, oc:oc + 1], None, op0=OP.add)
                _lrelu_bias(nc, scr_pool, ps[:], xo[:, 1:17, 1:17], gb1s[:, oc:oc + 1], 128)
                nc.sync.dma_start(t["x1_d"][128 * oc:128 * (oc + 1)].rearrange(
                    "c a b -> c (a b)"), xo[:].rearrange("p a b -> p (a b)"))

                # g2: 1x1 on a3 interior -> x2 (34x34 padded output)
                gt2 = wpool.tile([128, 2, 128], CDT, name=f"g2t{oc}", tag="g2t")
                nc.sync.dma_start(gt2[:], t["dg2_d"][:, oc].bitcast(CDT))
                xo2 = bounce_pool.tile([128, 34, 34], f32, name=f"x2b{oc}", tag="x2b")
                nc.vector.memset(xo2[:], 0.0)
                nc.vector.tensor_scalar(xo2[:], xo2[:], bv[:, 1, oc:oc + 1], None, op0=OP.add)
                for nh in range(2):
                    ps = ps_main.tile([128, 16, 32], f32, name=f"g2p{oc}{nh}", tag="psM")
                    for k in range(2):
                        nc.tensor.matmul(ps[:], gt2[:, k, :],
                                         a3[:, k, 1 + 16 * nh:17 + 16 * nh, 1:33],
                                         start=(k == 0), stop=(k == 1))
                    _lrelu_bias(nc, scr_pool, ps[:], xo2[:, 1 + 16 * nh:17 + 16 * nh, 1:33],
                                gb2s[:, oc:oc + 1], 128)
                nc.sync.dma_start(t["x2_d"][128 * oc:128 * (oc + 1)].rearrange(
                    "c a b -> c (a b)"), xo2[:].rearrange("p a b -> p (a b)"))



# ---------------------------------------------------------------- entry
def kernel(real_image, input_semantic, w1, w2, w3, w4, w5, w6, gw1, gb1, gw2, gb2):
    global LAST_EXEC_NS, LAST_RESULTS
    real_image = np.asarray(real_image, np.float32)
    input_semantic = np.asarray(input_semantic, np.float32)

    if "nc" not in _CACHE:
        _CACHE["nc"] = _build()
    nc = _CACHE["nc"]

    d1, d2, d3, d4, d5, d6 = _pack_weights([np.asarray(w, np.float32)
                                            for w in (w1, w2, w3, w4, w5, w6)])
    dg1 = _pack_g(np.asarray(gw1, np.float32), 4)
    dg2 = _pack_g(np.asarray(gw2, np.float32), 2)
    dgb1 = np.asarray(gb1, np.float32).reshape(16, 128).T.copy()
    dgb2 = np.asarray(gb2, np.float32).reshape(16, 128).T.copy()

    cones = np.ones((128, 1), np.float32)
    crow = np.ones((1, 128), np.float32)
    cid = np.eye(128, dtype=np.float32)
    ciota = np.broadcast_to(np.arange(256, dtype=np.float32), (128, 256)).copy()
    c256 = np.stack([np.arange(128, dtype=np.float32) * 256,
                     (np.arange(128, dtype=np.float32) + 128) * 256], 1).copy()

    shared = dict(d1=d1, d2=d2, d3=d3, d4=d4, d5=d5, d6=d6, dg1=dg1, dg2=dg2,
                  gb1=dgb1, gb2=dgb2, cones=cones, crow=crow, cid=cid,
                  ciota=ciota, c256=c256)
    in_maps = [dict(img=real_image[b], sem=input_semantic[b], **shared)
               for b in range(B)]

    trace = bool(os.environ.get("BASS_TRACE_RUN"))
    if trace:
        import ntff_shim
        ntff_shim.install()
    res = run_bass_kernel_spmd(nc, in_maps, core_ids=list(range(8)), trace=trace)
    LAST_EXEC_NS = res.exec_time_ns
    LAST_RESULTS = res
    x = np.stack([res.results[b]["x"] for b in range(B)])
    x1 = np.stack([res.results[b]["x1"] for b in range(B)])
    x2 = np.stack([res.results[b]["x2"] for b in range(B)])
    images = np.stack([res.results[b]["images"] for b in range(B)])
    return x, x1, x2, images
